# revision 1
# baseline (speedup 1.0000x reference)
"""Trainium2 Bass kernel for nn_MultiHeadAttention_78864189489198.

Reference computes (per batch b): q = x @ Wq; qh = heads(q);
scores = qh @ qh^T / sqrt(HD); attn = softmax(scores); y = attn @ qh;
out = merge(y) @ Wo + bo.   (q, k, v all use Wq -> scores are SYMMETRIC.)

Sharding (8 cores): core = b*4 + hg handles batch b and head-group hg
(4 heads = 512 of Wq columns / Wo rows). Each core computes a partial
out[b] contribution; host sums the 4 partials per batch and adds bo
(tensor-parallel all-reduce done on host since full I/O is host-side).

Per-core dataflow (all matmuls contract over the partition dim):
  x^T (host-transposed) + Wq slice --f32r MM--> QT [512, 2048] (heads x tok)
  QT -PE transpose-> Qnat fp16 [tok, 512]
  per head c: S tile (i-chunk a) = QT[c,a-chunk]^T MM QT[c,:]  (f32r)
    exp via ACT: es_a = exp(S*scale - C) -> fp16, accum_out = rowsum r
    (symmetry: es_a rows serve as P^T columns for the PV matmul)
    PV: YT_unnorm[d, i] = sum_a Qnat[a,c]^T MM es_a   (fp16)
    normalize by 1/r[i] along free axis (PE-free: transpose r via PE,
    flatten via DMA, broadcast via GpSimd partition_broadcast)
  out-proj: out[i,:] = sum_c YT[c, i-chunk]^T MM Wo[c,:]  (f32r)
"""

import os

import numpy as np

import concourse.bass as bass
import concourse.mybir as mybir
import concourse.tile as tile
from concourse import bacc
from concourse.bass_utils import run_bass_kernel_spmd
from concourse.masks import make_identity
from contextlib import ExitStack

P = 128
N = 2048          # tokens
D = 2048          # model dim
KO = D // P       # 16 contraction chunks
HG = 4            # heads per core
HD = 128          # head dim
HCOLS = HG * HD   # 512 q-columns per core
SP = N // 512     # 4 spans of 512 tokens
NCH = N // P      # 16 token chunks
SCALE = HD ** -0.5
C_BIAS = 9.0      # exp(S*SCALE - C) keeps fp16 in range (max scaled S ~19.4)

f32 = mybir.dt.float32
f32r = mybir.dt.float32r
f16 = mybir.dt.float16

_CACHE = {}


def build_nc():
    nc = bacc.Bacc("TRN2", target_bir_lowering=False, debug=False)
    xt = nc.dram_tensor("xt", [D, N], f32r, kind="ExternalInput")
    wq = nc.dram_tensor("wq", [D, HCOLS], f32r, kind="ExternalInput")
    wo = nc.dram_tensor("wo", [HCOLS, D], f32r, kind="ExternalInput")
    out = nc.dram_tensor("out", [N, D], f32, kind="ExternalOutput")

    xt3 = xt.rearrange("(ko p) n -> p ko n", p=P)      # [128, 16, 2048]
    wq3 = wq.rearrange("(ko p) m -> p ko m", p=P)      # [128, 16, 512]
    wo3 = wo.rearrange("(c p) n -> p c n", p=P)        # [128, 4, 2048]
    out3 = out.rearrange("(a p) n -> p a n", p=P)      # [128, 16, 2048]

    with (
        nc.allow_low_precision(reason="f32r intermediates are intentional"),
        tile.TileContext(nc) as tc,
        ExitStack() as ctx,
    ):
        # ---- persistent pools; psum pools are global so no phase ever
        # serializes on psum bank reuse across stage boundaries ----
        const_pool = ctx.enter_context(tc.tile_pool(name="const", bufs=1))
        qt_pool = ctx.enter_context(tc.tile_pool(name="qt", bufs=1))
        qn_pool = ctx.enter_context(tc.tile_pool(name="qn", bufs=1))
        es_pool = ctx.enter_context(tc.tile_pool(name="es", bufs=17))
        ps_a = ctx.enter_context(tc.tile_pool(name="ps_a", bufs=2, space="PSUM"))
        ps_b = ctx.enter_context(tc.tile_pool(name="ps_b", bufs=4, space="PSUM"))

        ident_r = const_pool.tile([P, P], f32r, tag="ident")
        cbias = const_pool.tile([P, 1], f32, tag="cbias")
        nc.gpsimd.memset(cbias[:], -C_BIAS)
        ident = ident_r[:]

        qt_sb = qt_pool.tile([P, HG, N], f32r, tag="qt")    # QT: [hd, c, tok]
        qn_sb = qn_pool.tile([P, NCH, HCOLS], f16, tag="qn")  # Qnat: [tokP, a, qcol]

        # ================= Stage A: load + Q projection =================
        # 4 quarter-K passes keep the x^T working set at 6 tiles so the
        # attention pools never alias it (enables phase overlap).  The
        # last pass emits only c=0 inline; the rest of the Qproj tail and
        # all Qnat transposes are interleaved into head 0's chunk loop so
        # ACT (exp) starts as early as possible.
        NPASS, KQ = 4, 4
        rr_pool = ctx.enter_context(tc.tile_pool(name="rr", bufs=2))

        def scores_chunk(c, a, rrech, es_tiles):
            es = es_pool.tile([P, N], f16, tag="es")
            for h2 in range(2):
                ps = ps_a.tile([P, 1024], f32, tag="s")
                for b2 in range(2):
                    b4 = h2 * 2 + b2
                    nc.tensor.matmul(
                        ps[:, b2 * 512:(b2 + 1) * 512],
                        qt_sb[:, c, a * P:(a + 1) * P],
                        qt_sb[:, c, b4 * 512:(b4 + 1) * 512],
                        start=True,
                        stop=True,
                    )
                nc.scalar.activation(
                    es[:, h2 * 1024:(h2 + 1) * 1024],
                    ps[:],
                    mybir.ActivationFunctionType.Exp,
                    bias=cbias[:, 0:1],
                    scale=SCALE,
                    accum_out=rrech[:, a, h2:h2 + 1],
                )
            es_tiles.append(es)

        def r_chain(c, rrech):
            # r = half0 + half1; then 1/r, transposed to a [1, N] row,
            # broadcast to all partitions
            rrec = rr_pool.tile([P, NCH], f32, tag="rrec")
            nc.vector.tensor_reduce(
                rrec[:],
                rrech[:],
                mybir.AxisListType.X,
                mybir.AluOpType.add,
            )
            rrec2 = rr_pool.tile([P, NCH], f32r, tag="rrec2")
            nc.vector.reciprocal(rrec2[:], rrec[:])
            prt = ps_b.tile([NCH, P], f32r, tag="b")
            nc.tensor.transpose(prt[:], rrec2[:], ident)
            rt16 = rr_pool.tile([NCH, P], f32r, tag="rt16")
            nc.vector.tensor_copy(rt16[:], prt[:])
            rbc = rbc_pool.tile([P, N], f32r, tag="rbc")
            nc.sync.dma_start(rbc[0:1, :], rt16[:, :])
            for b4 in range(SP):
                # per-span broadcast so the later per-span norms (and the
                # final head's out-proj) can start before the whole row
                # is broadcast
                nc.gpsimd.partition_broadcast(
                    rbc[:, b4 * 512:(b4 + 1) * 512],
                    rbc[0:1, b4 * 512:(b4 + 1) * 512],
                )
            return rbc

        def scores_part(c, a, h2, rrech, es):
            # one 1024-wide half of a scores chunk: 2 MMs + 1 exp
            ps = ps_a.tile([P, 1024], f32, tag="s")
            for b2 in range(2):
                b4 = h2 * 2 + b2
                nc.tensor.matmul(
                    ps[:, b2 * 512:(b2 + 1) * 512],
                    qt_sb[:, c, a * P:(a + 1) * P],
                    qt_sb[:, c, b4 * 512:(b4 + 1) * 512],
                    start=True,
                    stop=True,
                )
            nc.scalar.activation(
                es[:, h2 * 1024:(h2 + 1) * 1024],
                ps[:],
                mybir.ActivationFunctionType.Exp,
                bias=cbias[:, 0:1],
                scale=SCALE,
                accum_out=rrech[:, a, h2:h2 + 1],
            )

        rrech0 = rr_pool.tile([P, NCH, 2], f32, tag="rrech")
        es0_tiles = []
        with (
            tc.tile_pool(name="xt", bufs=24) as xt_pool,
            tc.tile_pool(name="wq", bufs=1) as wq_pool,
            tc.tile_pool(name="identf", bufs=1) as identf_pool,
        ):
            ident_f = identf_pool.tile([P, P], f32, tag="ident_f")
            make_identity(nc, ident_f[:])
            nc.vector.tensor_copy(ident_r[:], ident_f[:])
            # preload the exp table set so the first real exp doesn't pay it
            edum = identf_pool.tile([P, 1], f32, tag="edum")
            nc.scalar.activation(
                edum[:], cbias[:, 0:1], mybir.ActivationFunctionType.Exp
            )

            wq_sb = wq_pool.tile([P, KO, HCOLS], f32r, tag="wq")
            nc.gpsimd.dma_start(wq_sb[:], wq3[:])

            # x^T streamed in t-major 512-column pieces: all 16 chunks of
            # token-span t land before span t+1, so qt c0 spans complete
            # progressively and head 0's first exps start ~30us earlier.
            pieces = {}
            for t in range(SP):
                for ko in range(KO):
                    pc = xt_pool.tile([P, 512], f32r, tag="xt", name=f"x{ko}_{t}")
                    # sync+gpsimd only: ACT's queue must stay free of
                    # DMA issues or the exp stream stalls behind them
                    eng = nc.sync if (ko + t) % 2 == 0 else nc.gpsimd
                    eng.dma_start(pc[:], xt3[:, ko, t * 512:(t + 1) * 512])
                    pieces[(ko, t)] = pc

            def qproj_chain(pss, c, t):
                ps = ps_b.tile([P, 512], f32, tag="b")
                for kq in range(KQ):
                    ko = pss * KQ + kq
                    nc.tensor.matmul(
                        ps[:],
                        wq_sb[:, ko, c * P:(c + 1) * P],
                        pieces[(ko, t)][:],
                        start=(kq == 0),
                        stop=(kq == KQ - 1),
                    )
                dst = qt_sb[:, c, t * 512:(t + 1) * 512]
                if pss == 0:
                    nc.vector.tensor_copy(dst, ps[:])
                else:
                    nc.vector.tensor_tensor(
                        dst, ps[:], dst, mybir.AluOpType.add
                    )

            def transpose_j(c, j):
                # Qnat via PE transpose of a QT block
                pt = ps_b.tile([P, P], f32r, tag="b")
                nc.tensor.transpose(
                    pt[:], qt_sb[:, c, j * P:(j + 1) * P], ident
                )
                nc.vector.tensor_copy(qn_sb[:, j, c * P:(c + 1) * P], pt[:])

            # Qproj chains in t-major order, matched to the DMA stream
            for t in range(2):
                for pss in range(NPASS):
                    for c in range(HG):
                        qproj_chain(pss, c, t)

            # phase 1: head-0 exps that need only qt c0 spans 0-1
            # (h2=0 -> rhs cols [0:1024]; lhsT chunk a<8 -> spans 0-1)
            for a in range(8):
                es = es_pool.tile([P, N], f16, tag="es")
                scores_part(0, a, 0, rrech0, es)
                es0_tiles.append(es)

            # t2 fully inline (releases its x pieces); for t3 only c0 is
            # inline -- heads 1-3's t3 chains and all Qnat transposes are
            # interleaved into phase 2
            for pss in range(NPASS):
                for c in range(HG):
                    qproj_chain(pss, c, 2)
            for pss in range(NPASS):
                qproj_chain(pss, 0, 3)

            tailq = []
            for j in range(NCH):
                tailq.append(lambda j=j: transpose_j(0, j))
            for c in range(1, HG):
                for pss in range(NPASS):
                    tailq.append(lambda p=pss, c=c: qproj_chain(p, c, 3))
                for j in range(NCH):
                    tailq.append(lambda c=c, j=j: transpose_j(c, j))

            # phase 2: the rest of head 0, interleaved with the tail work
            for a in range(NCH):
                if a < 8:
                    scores_part(0, a, 1, rrech0, es0_tiles[a])
                else:
                    es = es_pool.tile([P, N], f16, tag="es")
                    scores_part(0, a, 0, rrech0, es)
                    scores_part(0, a, 1, rrech0, es)
                    es0_tiles.append(es)
                npop = (len(tailq) + NCH - 1 - a) // (NCH - a)
                for _ in range(npop):
                    if tailq:
                        tailq.pop(0)()
            while tailq:
                tailq.pop(0)()

        # ============ Stage B/C: attention + output projection ============
        with (
            tc.tile_pool(name="yt", bufs=1) as yt_pool,
            tc.tile_pool(name="wo", bufs=1) as wo_pool,
            tc.tile_pool(name="rbc", bufs=2) as rbc_pool,
            tc.tile_pool(name="osb", bufs=4) as o_pool,
        ):
            yt_sb = yt_pool.tile([P, HG, N], f32r, tag="yt")  # [hd, c, tok]
            wo_sb = wo_pool.tile([P, HG, D], f32r, tag="wo")
            nc.gpsimd.dma_start(wo_sb[:], wo3[:])

            def emit_norm(c, rbc):
                for b4 in range(SP):
                    nc.vector.tensor_tensor(
                        yt_sb[:, c, b4 * 512:(b4 + 1) * 512],
                        yt_sb[:, c, b4 * 512:(b4 + 1) * 512],
                        rbc[:, b4 * 512:(b4 + 1) * 512],
                        mybir.AluOpType.mult,
                    )

            rbc0 = r_chain(0, rrech0)
            prev = (0, es0_tiles, rbc0)

            for c in range(1, HG + 1):
                # PV chains of head c-1 run concurrently (one psum bank
                # per output span), grouped per contraction chunk a so
                # each es tile's last read happens as early as possible
                # and the es slots recycle rolling.
                pc, pes = prev[0], prev[1]
                psy = [
                    ps_b.tile([P, 512], f32, tag="b", name=f"psy{b4}")
                    for b4 in range(SP)
                ]
                rrech = None
                es_tiles = []
                if c < HG:
                    rrech = rr_pool.tile([P, NCH, 2], f32, tag="rrech")
                for a in range(NCH):
                    if c < HG:
                        scores_chunk(c, a, rrech, es_tiles)
                    for b4 in range(SP):
                        nc.tensor.matmul(
                            psy[b4][:],
                            qn_sb[:, a, pc * P:(pc + 1) * P],
                            pes[a][:, b4 * 512:(b4 + 1) * 512],
                            start=(a == 0),
                            stop=(a == NCH - 1),
                        )
                for b4 in range(SP):
                    # evacuate unnormalized; normalized in place below so
                    # PE/psum never wait on the 1/r broadcast chain
                    nc.vector.tensor_copy(
                        yt_sb[:, pc, b4 * 512:(b4 + 1) * 512], psy[b4][:]
                    )
                emit_norm(pc, prev[2])
                if c < HG:
                    prev = (c, es_tiles, r_chain(c, rrech))

            # ---------------- output projection ----------------
            for a in range(NCH):
                for d4 in range(SP):
                    ps = ps_b.tile([P, 512], f32, tag="b")
                    for c in range(HG):
                        nc.tensor.matmul(
                            ps[:],
                            yt_sb[:, c, a * P:(a + 1) * P],
                            wo_sb[:, c, d4 * 512:(d4 + 1) * 512],
                            start=(c == 0),
                            stop=(c == HG - 1),
                        )
                    ot = o_pool.tile([P, 512], f32, tag="ot")
                    if (a * SP + d4) % 2 == 0:
                        nc.vector.tensor_copy(ot[:], ps[:])
                    else:
                        nc.scalar.copy(ot[:], ps[:])
                    nc.sync.dma_start(
                        out3[:, a, d4 * 512:(d4 + 1) * 512], ot[:]
                    )

    nc.compile()
    return nc


def kernel(x, Wq, Wo, bo):
    x = np.asarray(x)
    Wq = np.asarray(Wq)
    Wo = np.asarray(Wo)
    bo = np.asarray(bo)
    B = x.shape[0]
    assert B == 2, "sharding hardcodes B=2 (core = b*4 + head_group)"
    assert x.shape == (B, N, D) and Wq.shape == (D, D) and Wo.shape == (D, D)

    if "nc" not in _CACHE:
        _CACHE["nc"] = build_nc()
    nc = _CACHE["nc"]

    xts = [np.ascontiguousarray(x[b].T).astype(np.float32) for b in range(B)]
    in_maps = []
    for core in range(8):
        b, hg = core // 4, core % 4
        in_maps.append(
            {
                "xt": xts[b],
                "wq": np.ascontiguousarray(
                    Wq[:, hg * HCOLS:(hg + 1) * HCOLS]
                ).astype(np.float32),
                "wo": np.ascontiguousarray(
                    Wo[hg * HCOLS:(hg + 1) * HCOLS, :]
                ).astype(np.float32),
            }
        )

    trace = bool(os.environ.get("KERNEL_TRACE"))
    try:
        res = run_bass_kernel_spmd(nc, in_maps, list(range(8)), trace=trace)
    except ModuleNotFoundError:
        # NTFF profiling hooks unavailable under this axon container
        res = run_bass_kernel_spmd(nc, in_maps, list(range(8)))
    _CACHE["last_res"] = res
    out = np.zeros((B, N, D), dtype=np.float32)
    for core in range(8):
        b = core // 4
        out[b] += res.results[core]["out"]
    out += bo.astype(np.float32)
    return out

